# revision 1
# baseline (speedup 1.0000x reference)
"""Trainium2 Bass kernel for linear-chain CRF forward algorithm (log partition).

Problem: input_features [2048, 512, 32] f32, transitions [32, 32] f32
         -> log Z [2048] f32.

Data-parallel over batch: 8 cores x 256 batch rows.  Per core:

  Probability-space scan  P_{t+1} = (W @ P_t) * exp(e_t - MU), with
  W = exp(transitions).  State P is [128, 64] bf16: tags on partitions in 4
  block-diagonal bands (one per 64-batch group), batch columns in the free
  dim.  Per step: one PE matmul (block-diagonal exp(transitions), bf16) into
  PSUM, one DVE tensor-tensor multiply with the emission factor back to SBUF.
  Every 128 steps the state is renormalized per batch column; the log of the
  norm is accumulated (ACT Ln/Exp pair - one shared table set).

  Emissions stream in via gpsimd cast-DMAs (fp32 HBM -> bf16 SBUF, natural
  [t, i] layout, contiguous reads), are transposed to the [(g,i), c, t] scan
  layout with PE matmuls against an identity (4 col-groups via tile_position),
  and exponentiated by ACT on the PSUM->SBUF copy (bias = -MU folded in).
  Transposes for chunks 1-3 are interleaved into the scan's PE dead time.

  Raw bass (no Tile): each instruction carries at most one attached wait and
  one semaphore update, matching the ISA EVENTS encoding this toolchain's
  walrus accepts.
"""

import os
import sys
import numpy as np

for _p in ("/opt/trn_rl_repo",):
    if _p not in sys.path and os.path.isdir(_p):
        sys.path.insert(0, _p)

import ml_dtypes

B, S, T = 2048, 512, 32
START_TAG, STOP_TAG = 30, 31
NCORES = 8
BL = B // NCORES          # 256 batch rows per core
G = 4                     # partition bands (batch groups)
C = BL // G               # 64 batch columns per band
CHUNK = 128               # scan steps per emission tile
NCHUNK = S // CHUNK
CQ = 4                    # batch columns per transpose quad / PSUM tile
NQ = C // CQ              # quads per chunk (16)
MU = 4.4                  # per-step growth estimate subtracted from emissions
RENORM_TS = (64, 192, 320, 448)
# chunk-k transposes are paced across the scan of chunk k-1: 3 per step for
# the first 16 steps, 2 per step through local step 120 (3*16+2*104 = 256).

_cache = {}


def _build_program():
    """Build the raw-bass program (shared SPMD across all 8 cores)."""
    from concourse import bass, mybir

    f32 = mybir.dt.float32
    bf16 = mybir.dt.bfloat16
    AF = mybir.ActivationFunctionType

    nc = bass.Bass("TRN2", target_bir_lowering=False, debug=False)

    emis = nc.dram_tensor("emis", [BL, S, T], f32, kind="ExternalInput").ap()
    wbd_d = nc.dram_tensor("wbd", [128, 128], bf16, kind="ExternalInput").ap()
    wstop_d = nc.dram_tensor("wstop", [128, G], bf16, kind="ExternalInput").ap()
    ones_d = nc.dram_tensor("onesbd", [128, G], bf16, kind="ExternalInput").ap()
    ind_d = nc.dram_tensor("ind", [G, 128], f32, kind="ExternalInput").ap()
    wstart_d = nc.dram_tensor("wstart", [128, 1], f32, kind="ExternalInput").ap()
    ident_d = nc.dram_tensor("ident", [128, 128], bf16, kind="ExternalInput").ap()
    bmu_d = nc.dram_tensor("bmu", [128, 1], f32, kind="ExternalInput").ap()
    z4_d = nc.dram_tensor("z4", [G, 1], f32, kind="ExternalInput").ap()
    outp = nc.dram_tensor("outp", [G, C], f32, kind="ExternalOutput").ap()

    # SBUF
    wbd_s = nc.alloc_sbuf_tensor("wbd_s", [128, 128], bf16).ap()
    wstop_s = nc.alloc_sbuf_tensor("wstop_s", [128, G], bf16).ap()
    ones_s = nc.alloc_sbuf_tensor("ones_s", [128, G], bf16).ap()
    ind_s = nc.alloc_sbuf_tensor("ind_s", [G, 128], f32).ap()
    wstart_s = nc.alloc_sbuf_tensor("wstart_s", [128, 1], f32).ap()
    ident_s = nc.alloc_sbuf_tensor("ident_s", [128, 128], bf16).ap()
    bmu_s = nc.alloc_sbuf_tensor("bmu_s", [128, 1], f32).ap()
    z4_s = nc.alloc_sbuf_tensor("z4_s", [G, 1], f32).ap()
    gt = [
        nc.alloc_sbuf_tensor(f"gt{g}", [128, NCHUNK, C, T], bf16).ap()
        for g in range(G)
    ]
    em = [
        nc.alloc_sbuf_tensor(f"em{i}", [128, C, CHUNK], bf16).ap() for i in range(2)
    ]
    Pst = [nc.alloc_sbuf_tensor(f"P{i}", [128, C], bf16).ap() for i in range(2)]
    acc = nc.alloc_sbuf_tensor("acc", [G, C], f32).ap()
    ls_s = nc.alloc_sbuf_tensor("ls_s", [G, C], f32).ap()
    v_s = nc.alloc_sbuf_tensor("v_s", [G, C], f32).ap()
    r1_s = nc.alloc_sbuf_tensor("r1_s", [G, C], f32).ap()
    res_s = nc.alloc_sbuf_tensor("res_s", [G, C], f32).ap()

    # PSUM: separate tensors -> separate banks (PE-writer vs reader safety)
    q2 = [nc.alloc_psum_tensor(f"q{i}", [128, C], f32).ap() for i in range(2)]
    tr2 = [
        nc.alloc_psum_tensor(f"tr{i}", [128, CQ, CHUNK], f32).ap() for i in range(2)
    ]
    s_ps = nc.alloc_psum_tensor("s_ps", [G, C], f32).ap()
    bc_ps = nc.alloc_psum_tensor("bc_ps", [128, C], f32).ap()

    consts = [
        (wbd_s, wbd_d), (wstop_s, wstop_d), (ones_s, ones_d), (ind_s, ind_d),
        (wstart_s, wstart_d), (ident_s, ident_d), (bmu_s, bmu_d), (z4_s, z4_d),
    ]
    CSEM_ALL = 16 * len(consts)

    # transpose emission schedule: per-MM records
    # record = (k, jj, j, ci, g, first_of_quad, last_of_quad, first_of_chunk)
    def quad_records(k, j):
        jj = k * NQ + j
        recs = []
        for ci in range(CQ):
            for g in range(G):
                recs.append(
                    (k, jj, j, ci, g, ci == 0 and g == 0,
                     ci == CQ - 1 and g == G - 1, j == 0 and ci == 0 and g == 0)
                )
        return recs

    tq = []
    for k in range(1, NCHUNK):
        for j in range(NQ):
            tq.extend(quad_records(k, j))

    import contextlib
    with contextlib.ExitStack() as st:
        csem = st.enter_context(nc.semaphore("csem"))
        ldsems = [
            st.enter_context(nc.semaphore(f"ld{k}_{ch}"))
            for k in range(NCHUNK) for ch in range(2)
        ]
        trq = st.enter_context(nc.semaphore("trq"))
        cps = st.enter_context(nc.semaphore("cps"))
        emf = st.enter_context(nc.semaphore("emf"))
        qs = st.enter_context(nc.semaphore("qs"))
        ps = st.enter_context(nc.semaphore("ps"))
        ps2 = st.enter_context(nc.semaphore("ps2"))
        ss = st.enter_context(nc.semaphore("ss"))
        lss = st.enter_context(nc.semaphore("lss"))
        vs = st.enter_context(nc.semaphore("vs"))
        bs = st.enter_context(nc.semaphore("bs"))
        fs = st.enter_context(nc.semaphore("fs"))
        osem = st.enter_context(nc.semaphore("osem"))

        with nc.Block() as blk:

            @blk.sync
            def _(e):
                for sb, dr in consts:
                    e.dma_start(out=sb, in_=dr).then_inc(csem, 16)
                e.wait_ge(fs, 1)
                e.dma_start(out=outp, in_=res_s).then_inc(osem, 16)
                e.wait_ge(osem, 16)

            @blk.gpsimd
            def _(e):
                # cast-DMAs fp32 -> bf16; k-major so chunk k is complete after
                # 8*(k+1) transfers
                for k in range(NCHUNK):
                    for ch in range(2):
                        for g in range(G):
                            c0 = ch * (C // 2)
                            e.dma_start(
                                out=gt[g][:, k, c0 : c0 + C // 2, :],
                                in_=emis[
                                    g * C + c0 : g * C + c0 + C // 2,
                                    k * CHUNK : (k + 1) * CHUNK,
                                    :,
                                ].rearrange("c t i -> t c i"),
                            ).then_inc(ldsems[k * 2 + ch], 16)

            @blk.tensor
            def _(e):
                def transpose_mm(rec):
                    k, jj, j, ci, g, first_q, last_q, first_c = rec
                    if first_c:
                        e.wait_ge(ldsems[k * 2], 64)
                    if first_q and j == NQ // 2:
                        e.wait_ge(ldsems[k * 2 + 1], 64)
                    if first_q and jj >= 2:
                        e.wait_ge(cps, jj - 1)
                    c = j * CQ + ci
                    inst = e.matmul(
                        tr2[jj % 2][32 * g : 32 * (g + 1), ci, :],
                        gt[g][:, k, c, :],
                        ident_s,
                        start=True,
                        stop=True,
                        tile_position=(0, 32 * g),
                    )
                    if last_q:
                        inst.then_inc(trq, 1)

                e.wait_ge(csem, CSEM_ALL)
                # chunk 0 transposes up front
                for j in range(NQ):
                    for rec in quad_records(0, j):
                        transpose_mm(rec)

                ti = 0
                r = 0
                for t in range(1, S):
                    e.matmul(
                        q2[t % 2], wbd_s, Pst[t % 2], start=True, stop=True
                    )._wait_ge(ps, t).then_inc(qs, 1)
                    lt = t % CHUNK
                    if lt == 0:
                        lt = CHUNK
                    n_tr = 3 if lt <= 16 else (2 if lt <= 120 else 0)
                    for _i in range(n_tr):
                        if ti < len(tq):
                            transpose_mm(tq[ti])
                            ti += 1
                    if t in RENORM_TS:
                        e.matmul(
                            s_ps, ones_s, Pst[(t + 1) % 2], start=True, stop=True
                        )._wait_ge(ps2, r + 1).then_inc(ss, 1)
                        e.matmul(
                            bc_ps, ind_s, v_s, start=True, stop=True
                        )._wait_ge(vs, r + 1).then_inc(bs, 1)
                        r += 1
                assert ti == len(tq), (ti, len(tq))
                e.matmul(
                    s_ps, wstop_s, Pst[S % 2], start=True, stop=True
                )._wait_ge(ps, S).then_inc(ss, 1)

            @blk.scalar
            def _(e):
                def cp(k, j):
                    jj = k * NQ + j
                    e.activation(
                        em[k % 2][:, j * CQ : (j + 1) * CQ, :],
                        tr2[jj % 2],
                        AF.Exp,
                        bias=bmu_s,
                    )._wait_ge(trq, jj + 1).then_inc(cps, 1)

                def renorm_act(r):
                    e.activation(ls_s, s_ps, AF.Ln, bias=z4_s)._wait_ge(
                        ss, r + 1
                    ).then_inc(lss, 1)
                    e.activation(
                        v_s, ls_s, AF.Exp, bias=z4_s, scale=-1.0
                    ).then_inc(vs, 1)

                e.wait_ge(csem, CSEM_ALL)
                for j in range(NQ):
                    cp(0, j)
                for j in range(NQ // 2):
                    cp(1, j)
                renorm_act(0)                    # t = 64
                for j in range(NQ // 2, NQ):
                    cp(1, j)
                e.wait_ge(emf, 1)
                for j in range(NQ // 2):
                    cp(2, j)
                renorm_act(1)                    # t = 192
                for j in range(NQ // 2, NQ):
                    cp(2, j)
                e.wait_ge(emf, 2)
                for j in range(NQ // 2):
                    cp(3, j)
                renorm_act(2)                    # t = 320
                for j in range(NQ // 2, NQ):
                    cp(3, j)
                renorm_act(3)                    # t = 448
                e.activation(ls_s, s_ps, AF.Ln, bias=z4_s)._wait_ge(ss, 5).then_inc(
                    lss, 1
                )

            @blk.vector
            def _(e):
                e.memset(acc, 0.0)
                e.wait_ge(csem, CSEM_ALL)
                e.wait_ge(cps, NQ)
                e.tensor_scalar_mul(Pst[1], em[0][:, :, 0], wstart_s).then_inc(ps, 1)
                r = 0
                for t in range(1, S):
                    k = t // CHUNK
                    tl = t % CHUNK
                    if tl == 0:
                        e.wait_ge(cps, NQ * (k + 1))
                    inst = e.tensor_mul(
                        Pst[(t + 1) % 2], q2[t % 2], em[k % 2][:, :, tl]
                    )
                    inst._wait_ge(qs, t)
                    if t in RENORM_TS:
                        inst.then_inc(ps2, 1)
                        e.tensor_add(acc, acc, ls_s)._wait_ge(lss, r + 1)
                        e.tensor_mul(
                            Pst[(t + 1) % 2], bc_ps, Pst[(t + 1) % 2]
                        )._wait_ge(bs, r + 1).then_inc(ps, 1)
                        r += 1
                    else:
                        inst.then_inc(ps, 1)
                    if tl == CHUNK - 1 and k < 2:
                        e.nop().then_inc(emf, 1)
                e.scalar_tensor_tensor(
                    res_s, ls_s, float(S) * MU, acc,
                    mybir.AluOpType.add, mybir.AluOpType.add,
                )._wait_ge(lss, 5).then_inc(fs, 1)

    return nc


def _host_consts(transitions):
    """Host-side tiny constant matrices (replicated per core)."""
    tr = np.asarray(transitions, np.float32)
    W = np.exp(tr)                      # W[i, j] = exp(trans[i, j])
    lhsT = W.T.copy()                   # lhsT[j, i]
    wbd = np.zeros((128, 128), np.float32)
    ones_bd = np.zeros((128, G), np.float32)
    wstop_bd = np.zeros((128, G), np.float32)
    ind = np.zeros((G, 128), np.float32)
    wstop_row = np.exp(tr[STOP_TAG, :])
    for g in range(G):
        wbd[32 * g : 32 * (g + 1), 32 * g : 32 * (g + 1)] = lhsT
        ones_bd[32 * g : 32 * (g + 1), g] = 1.0
        wstop_bd[32 * g : 32 * (g + 1), g] = wstop_row
        ind[g, 32 * g : 32 * (g + 1)] = 1.0
    wstart = np.tile(np.exp(tr[:, START_TAG]), G).reshape(128, 1)
    bf = ml_dtypes.bfloat16
    return {
        "wbd": wbd.astype(bf),
        "wstop": wstop_bd.astype(bf),
        "onesbd": ones_bd.astype(bf),
        "ind": ind.astype(np.float32),
        "wstart": wstart.astype(np.float32),
        "ident": np.eye(128, dtype=np.float32).astype(bf),
        "bmu": np.full((128, 1), -MU, np.float32),
        "z4": np.zeros((G, 1), np.float32),
    }


def _run(input_features, transitions, trace=False):
    from concourse import bass_utils

    feats = np.ascontiguousarray(np.asarray(input_features, np.float32))
    consts = _host_consts(transitions)

    if "nc" not in _cache:
        _cache["nc"] = _build_program()
    nc = _cache["nc"]

    in_maps = []
    for c in range(NCORES):
        m = dict(consts)
        m["emis"] = feats[c * BL : (c + 1) * BL]
        in_maps.append(m)

    res = bass_utils.run_bass_kernel_spmd(
        nc, in_maps, core_ids=list(range(NCORES)), trace=trace
    )
    out = np.concatenate(
        [np.asarray(res.results[c]["outp"], np.float32).reshape(BL) for c in range(NCORES)]
    )
    return out, res


def kernel(input_features, transitions):
    out, _ = _run(input_features, transitions, trace=False)
    return out



# revision 2
# speedup vs baseline: 10.8004x; 10.8004x over previous
"""Trainium2 Bass kernel for linear-chain CRF forward algorithm (log partition).

Bidirectional probability-space scan: the sequence is split at t=256;
a forward chain computes a = prod_{t=255..0}(diag(E_t) W) e_start and a
backward chain computes b = W^T diag(E_256) ... W^T diag(E_511) wstop;
log Z = log(sum_i a_i b_i) + renorm-log accumulators + S*MU.

Per core (256 batches): state tiles [128, 64] bf16, 4 partition bands of
32 tags; band r, column j <-> batch 4j+r. Per global step both chains do
one PE matmul (block-diag exp(transitions), resident-reloaded bf16
stationary) and one DVE multiply with the emission factor (PSUM -> SBUF).

Emissions stream in as natural-layout cast-DMAs (f32 HBM -> bf16 SBUF,
[t, c, i]), are transposed on PE via packed identity matmuls (stationary =
[128 t, 4 batches x 32 tags] slab -> [128, 128] PSUM tile), and
exponentiated by ACT on the PSUM->SBUF copy ([128, 512] instructions,
bias = -MU). Chunks 0 and 3 transpose during the DMA startup window;
chunks 1 and 2 are paced into the scan's PE dead time in phase 0.
"""

import os
import sys
import numpy as np

for _p in ("/opt/trn_rl_repo",):
    if _p not in sys.path and os.path.isdir(_p):
        sys.path.insert(0, _p)

import ml_dtypes

B, S, T = 2048, 512, 32
START_TAG, STOP_TAG = 30, 31
NCORES = 8
BL = B // NCORES          # 256 batch rows per core
G = 4                     # partition bands
C = BL // G               # 64 state columns (batch quads)
CHUNK = 128
NCHUNK = S // CHUNK       # 4
HALF = S // 2             # 256 scan steps per chain
MU = 4.4
FWD_RENORM = (64, 192)
BWD_RENORM = (96, 224)

_cache = {}


def _build_program(nphase=2):
    from concourse import bass, mybir
    half = 128 * nphase
    nck = 2 * nphase

    f32 = mybir.dt.float32
    bf16 = mybir.dt.bfloat16
    AF = mybir.ActivationFunctionType

    nc = bass.Bass("TRN2", target_bir_lowering=False, debug=False)

    emis = nc.dram_tensor("emis", [BL, S, T], f32, kind="ExternalInput").ap()
    wf_d = nc.dram_tensor("wf", [128, 128], bf16, kind="ExternalInput").ap()
    wb_d = nc.dram_tensor("wb", [128, 128], bf16, kind="ExternalInput").ap()
    wstart_d = nc.dram_tensor("wstart", [128, 1], f32, kind="ExternalInput").ap()
    wstop_d = nc.dram_tensor("wstop", [128, 1], f32, kind="ExternalInput").ap()
    ones_d = nc.dram_tensor("onesbd", [128, G], bf16, kind="ExternalInput").ap()
    ind_d = nc.dram_tensor("ind", [G, 128], f32, kind="ExternalInput").ap()
    ident_d = nc.dram_tensor("ident", [128, 128], bf16, kind="ExternalInput").ap()
    bmu_d = nc.dram_tensor("bmu", [128, 1], f32, kind="ExternalInput").ap()
    z4_d = nc.dram_tensor("z4", [G, 1], f32, kind="ExternalInput").ap()
    outp = nc.dram_tensor("outp", [G, C], f32, kind="ExternalOutput").ap()

    # SBUF
    wf_s = nc.alloc_sbuf_tensor("wf_s", [128, 128], bf16).ap()
    wb_s = nc.alloc_sbuf_tensor("wb_s", [128, 128], bf16).ap()
    wstart_s = nc.alloc_sbuf_tensor("wstart_s", [128, 1], f32).ap()
    wstop_s = nc.alloc_sbuf_tensor("wstop_s", [128, 1], f32).ap()
    ones_s = nc.alloc_sbuf_tensor("ones_s", [128, G], bf16).ap()
    ind_s = nc.alloc_sbuf_tensor("ind_s", [G, 128], f32).ap()
    ident_s = nc.alloc_sbuf_tensor("ident_s", [128, 128], bf16).ap()
    bmu_s = nc.alloc_sbuf_tensor("bmu_s", [128, 1], f32).ap()
    z4_s = nc.alloc_sbuf_tensor("z4_s", [G, 1], f32).ap()
    nat = [
        nc.alloc_sbuf_tensor(f"nat{i}", [128, BL, T], bf16).ap() for i in range(2)
    ]
    em = [
        nc.alloc_sbuf_tensor(f"em{k}", [128, CHUNK, C], bf16).ap()
        for k in range(NCHUNK)
    ]
    Pf = [nc.alloc_sbuf_tensor(f"Pf{i}", [128, C], bf16).ap() for i in range(2)]
    Pb = [nc.alloc_sbuf_tensor(f"Pb{i}", [128, C], bf16).ap() for i in range(2)]
    acc = nc.alloc_sbuf_tensor("acc", [G, C], f32).ap()
    ls_s = nc.alloc_sbuf_tensor("ls_s", [G, C], f32).ap()
    v_s = nc.alloc_sbuf_tensor("v_s", [G, C], f32).ap()
    tdot = nc.alloc_sbuf_tensor("tdot", [128, C], bf16).ap()
    res_s = nc.alloc_sbuf_tensor("res_s", [G, C], f32).ap()

    # PSUM (8 banks): 2 transpose banks + 4 scan q tiles + renorm pair
    trp = [
        nc.alloc_psum_tensor(f"trp{i}", [128, 4, CHUNK], f32).ap() for i in range(2)
    ]
    qf = [nc.alloc_psum_tensor(f"qf{i}", [128, C], f32).ap() for i in range(2)]
    qb = [nc.alloc_psum_tensor(f"qb{i}", [128, C], f32).ap() for i in range(2)]
    s_ps = nc.alloc_psum_tensor("s_ps", [G, C], f32).ap()
    bc_ps = nc.alloc_psum_tensor("bc_ps", [128, C], f32).ap()

    consts = [
        (wf_s, wf_d), (wb_s, wb_d), (wstart_s, wstart_d), (wstop_s, wstop_d),
        (ones_s, ones_d), (ind_s, ind_d), (ident_s, ident_d), (bmu_s, bmu_d),
        (z4_s, z4_d),
    ]
    CSEM_ALL = 16 * len(consts)

    # chunk processing order (DMA / transpose / copy): 0, 3, 1, 2
    CORDER = (0, 3, 1, 2)
    NQTR = 4              # DMA transfers per chunk (64-batch quarters)
    NJ = C                # 64 transpose matmuls per chunk (4-batch quads)
    NCP = NJ // 4         # 16 ACT copies per chunk

    # natural-buffer index per chunk
    natof = {0: 0, 3: 1, 1: 1, 2: 0}

    # transpose schedule during scan: chunk 1 at steps 25..88 (1/step),
    # chunk 2 at steps 89..120 (2/step)
    def scan_tr_count(t):
        if 25 <= t <= 88:
            return 1
        if 89 <= t <= 120:
            return 2
        return 0

    # renorm event ordering: (step, chain): ss/lss/vs/bs event index e=1..4
    fwd_rn = tuple(t for t in FWD_RENORM if t < 128 * nphase)
    bwd_rn = tuple(t for t in BWD_RENORM if t < 128 * nphase)
    EVENTS = sorted([(t, "f") for t in fwd_rn] + [(t, "b") for t in bwd_rn])
    EIDX = {ev: i + 1 for i, ev in enumerate(EVENTS)}

    import contextlib
    with contextlib.ExitStack() as st:
        csem = st.enter_context(nc.semaphore("csem"))
        ldsems = {
            (k, q): st.enter_context(nc.semaphore(f"ld{k}_{q}"))
            for k in range(NCHUNK)
            for q in range(NQTR)
        }
        trq = st.enter_context(nc.semaphore("trq"))
        cps = st.enter_context(nc.semaphore("cps"))
        qsf = st.enter_context(nc.semaphore("qsf"))
        psf = st.enter_context(nc.semaphore("psf"))
        qsb = st.enter_context(nc.semaphore("qsb"))
        psb = st.enter_context(nc.semaphore("psb"))
        ps2 = st.enter_context(nc.semaphore("ps2"))
        ss = st.enter_context(nc.semaphore("ss"))
        lss = st.enter_context(nc.semaphore("lss"))
        vs = st.enter_context(nc.semaphore("vs"))
        bs = st.enter_context(nc.semaphore("bs"))
        ds = st.enter_context(nc.semaphore("ds"))
        fs = st.enter_context(nc.semaphore("fs"))
        osem = st.enter_context(nc.semaphore("osem"))

        with nc.Block() as blk:

            @blk.sync
            def _(e):
                for sb, dr in consts:
                    e.dma_start(out=sb, in_=dr).then_inc(csem, 16)
                e.wait_ge(fs, 1)
                e.dma_start(out=outp, in_=res_s).then_inc(osem, 16)
                e.wait_ge(osem, 16)

            @blk.gpsimd
            def _(e):
                # cast-DMAs f32 -> bf16, natural [t, c, i] layout.
                # startup chunks 0,3 interleaved by quarter; then 1, then 2.
                def xfer(k, q):
                    c0 = q * 64
                    e.dma_start(
                        out=nat[natof[k]][:, c0 : c0 + 64, :],
                        in_=emis[
                            c0 : c0 + 64, k * CHUNK : (k + 1) * CHUNK, :
                        ].rearrange("c t i -> t c i"),
                    ).then_inc(ldsems[(k, q)], 16)

                for q in range(NQTR):
                    xfer(0, q)
                    xfer(3, q)
                if nphase == 2:
                    # chunk 1 overwrites nat[1] (chunk 3): wait its transposes
                    e.wait_ge(trq, 2 * NJ)
                    for q in range(NQTR):
                        xfer(1, q)
                    # chunk 2 overwrites nat[0]: transposes done earlier
                    for q in range(NQTR):
                        xfer(2, q)

            # global transpose counter layout: chunk CORDER[ci] occupies
            # trq range [ci*NJ, (ci+1)*NJ); copy i covers trq 4(i+1)
            @blk.tensor
            def _(e):
                def transpose_mm(ci, k, j):
                    jj = ci * NJ + j
                    if j % 16 == 0 and jj >= 8:
                        # bank-reuse wait can't share the instruction with
                        # the ld wait: carry it on a nop
                        e.nop()._wait_ge(cps, jj // 4 - 1)
                    inst = e.matmul(
                        trp[(jj // 4) % 2][:, j % 4, :],
                        nat[natof[k]][:, 4 * j : 4 * j + 4, :],
                        ident_s,
                        start=True,
                        stop=True,
                    )
                    if j % 16 == 0:
                        inst._wait_ge(ldsems[(k, j // 16)], 16)
                    elif j % 4 == 0 and jj >= 8:
                        # bank reuse: ACT copy of group jj//4 - 2 done
                        inst._wait_ge(cps, jj // 4 - 1)
                    inst.then_inc(trq, 1)

                e.wait_ge(csem, CSEM_ALL)
                # startup: chunks 0 and 3
                for ci, k in enumerate(CORDER[:2]):
                    for j in range(NJ):
                        transpose_mm(ci, k, j)

                # scan: fwd t = 1..255 matmuls, bwd tau = 1..256
                ti = 0          # 0..127 scan-interleaved transposes (c1, c2)
                for t in range(1, half + 1):
                    if t < half:
                        e.matmul(
                            qf[t % 2], wf_s, Pf[t % 2], start=True, stop=True
                        )._wait_ge(psf, t).then_inc(qsf, 1)
                    e.matmul(
                        qb[t % 2], wb_s, Pb[t % 2], start=True, stop=True
                    )._wait_ge(psb, t).then_inc(qsb, 1)
                    for _ in range(scan_tr_count(t) if nphase == 2 else 0):
                        ci = 2 + ti // NJ
                        transpose_mm(ci, CORDER[ci], ti % NJ)
                        ti += 1
                    ev = (
                        (t, "f") if t in fwd_rn
                        else (t, "b") if t in bwd_rn
                        else None
                    )
                    if ev is not None:
                        ee = EIDX[ev]
                        P = Pf if ev[1] == "f" else Pb
                        e.matmul(
                            s_ps, ones_s, P[(t + 1) % 2], start=True, stop=True
                        )._wait_ge(ps2, ee).then_inc(ss, 1)
                        e.matmul(
                            bc_ps, ind_s, v_s, start=True, stop=True
                        )._wait_ge(vs, ee).then_inc(bs, 1)
                assert nphase == 1 or ti == 2 * NJ, ti
                # finale: dot reduction
                e.matmul(
                    s_ps, ones_s, tdot, start=True, stop=True
                )._wait_ge(ds, 1).then_inc(ss, 1)

            @blk.scalar
            def _(e):
                def cp(i, k):
                    # copy i covers trp group i (4 j's) of chunk k
                    b = (i % NCP) * 4
                    e.activation(
                        em[k][:, :, b : b + 4],
                        trp[i % 2].rearrange("p j t -> p t j"),
                        AF.Exp,
                        bias=bmu_s,
                    )._wait_ge(trq, 4 * (i + 1)).then_inc(cps, 1)

                def renorm_act(ee):
                    e.activation(ls_s, s_ps, AF.Ln, bias=z4_s)._wait_ge(
                        ss, ee
                    ).then_inc(lss, 1)
                    e.activation(
                        v_s, ls_s, AF.Exp, bias=z4_s, scale=-1.0
                    ).then_inc(vs, 1)

                e.wait_ge(csem, CSEM_ALL)
                # copies: chunks 0,3 (i=0..31), then 1,2 interleaved with
                # renorm events by expected completion order
                for i in range(2 * NCP):
                    cp(i, CORDER[i // NCP])
                # chunk 1 copies: transposes at steps 25..88 -> trq crosses
                # 4(i+1) around step ~25+4*(i-32)... events f64 ~ after copy
                # i=41, b96 ~ after copy 47 (end of chunk 1)
                if nphase == 2:
                    for i in range(2 * NCP, 3 * NCP):
                        cp(i, CORDER[2])
                        if i == 2 * NCP + 9:
                            renorm_act(1)       # fwd t=64
                    for i in range(3 * NCP, 4 * NCP):
                        cp(i, CORDER[3])
                        if i == 3 * NCP + 3:
                            renorm_act(2)       # bwd t=96
                    renorm_act(3)               # fwd t=192
                    renorm_act(4)               # bwd t=224
                else:
                    renorm_act(1)
                    renorm_act(2)
                # finale
                e.activation(ls_s, s_ps, AF.Ln, bias=z4_s)._wait_ge(
                    ss, len(EVENTS) + 1
                ).then_inc(lss, 1)

            @blk.vector
            def _(e):
                e.wait_ge(csem, CSEM_ALL)
                e.memset(acc, 0.0)
                # init: a_1 = E_0 * wstart ; u_511 = E_511 * wstop
                e.tensor_scalar_mul(
                    Pf[1], em[0][:, 0, :], wstart_s
                )._wait_ge(cps, NCP).then_inc(psf, 1)
                e.tensor_scalar_mul(
                    Pb[1], em[3][:, CHUNK - 1, :], wstop_s
                )._wait_ge(cps, 2 * NCP).then_inc(psb, 1)

                for t in range(1, half):
                    ev = (
                        (t, "f") if t in fwd_rn
                        else (t, "b") if t in bwd_rn
                        else None
                    )
                    # fwd multiply, consumes E_t
                    k, tl = t // CHUNK, t % CHUNK
                    if t == CHUNK and nphase == 2:
                        e.nop()._wait_ge(cps, 3 * NCP)
                    instf = e.tensor_mul(
                        Pf[(t + 1) % 2], qf[t % 2], em[k][:, tl, :]
                    )
                    instf._wait_ge(qsf, t)
                    # bwd multiply, consumes E_{511-t}
                    tb = S - 1 - t
                    kb, tlb = tb // CHUNK, tb % CHUNK
                    if t == CHUNK and nphase == 2:
                        e.nop()._wait_ge(cps, 4 * NCP)
                    instb = e.tensor_mul(
                        Pb[(t + 1) % 2], qb[t % 2], em[kb][:, tlb, :]
                    )
                    instb._wait_ge(qsb, t)
                    if ev is not None:
                        ee = EIDX[ev]
                        if ev[1] == "f":
                            inst, other, P, sem, osem2 = instf, instb, Pf, psf, psb
                        else:
                            inst, other, P, sem, osem2 = instb, instf, Pb, psb, psf
                        inst.then_inc(ps2, 1)
                        other.then_inc(osem2, 1)
                        e.tensor_add(acc, acc, ls_s)._wait_ge(lss, ee)
                        e.tensor_mul(
                            P[(t + 1) % 2], bc_ps, P[(t + 1) % 2]
                        )._wait_ge(bs, ee).then_inc(sem, 1)
                    else:
                        instf.then_inc(psf, 1)
                        instb.then_inc(psb, 1)
                # bwd final multiply tau=255 consumed E_256 above; one more
                # matmul (tau=256) produced b in qb[0]; fwd state a in Pf[0]
                e.tensor_mul(tdot, qb[0], Pf[0])._wait_ge(qsb, half).then_inc(
                    ds, 1
                )
                e.scalar_tensor_tensor(
                    res_s, ls_s, float(S) * MU, acc,
                    mybir.AluOpType.add, mybir.AluOpType.add,
                )._wait_ge(lss, len(EVENTS) + 1).then_inc(fs, 1)

    return nc


def _host_consts(transitions):
    tr = np.asarray(transitions, np.float32)
    W = np.exp(tr)                      # W[i, j] = exp(trans[i, j])
    lhsT_f = W.T.copy()                 # fwd stationary block: out = W @ P
    lhsT_b = W.copy()                   # bwd stationary block: out = W.T @ u
    wf = np.zeros((128, 128), np.float32)
    wb = np.zeros((128, 128), np.float32)
    ones_bd = np.zeros((128, G), np.float32)
    ind = np.zeros((G, 128), np.float32)
    for g in range(G):
        sl = slice(32 * g, 32 * (g + 1))
        wf[sl, sl] = lhsT_f
        wb[sl, sl] = lhsT_b
        ones_bd[sl, g] = 1.0
        ind[g, sl] = 1.0
    wstart = np.tile(np.exp(tr[:, START_TAG]), G).reshape(128, 1)
    wstop = np.tile(np.exp(tr[STOP_TAG, :]), G).reshape(128, 1)
    bf = ml_dtypes.bfloat16
    return {
        "wf": wf.astype(bf),
        "wb": wb.astype(bf),
        "wstart": wstart.astype(np.float32),
        "wstop": wstop.astype(np.float32),
        "onesbd": ones_bd.astype(bf),
        "ind": ind.astype(np.float32),
        "ident": np.eye(128, dtype=np.float32).astype(bf),
        "bmu": np.full((128, 1), -MU, np.float32),
        "z4": np.zeros((G, 1), np.float32),
    }


def _run(input_features, transitions, trace=False, nphase=2):
    from concourse import bass_utils

    feats = np.ascontiguousarray(np.asarray(input_features, np.float32))
    consts = _host_consts(transitions)

    key = f"nc{nphase}"
    if key not in _cache:
        _cache[key] = _build_program(nphase)
    nc = _cache[key]

    in_maps = []
    for c in range(NCORES):
        m = dict(consts)
        m["emis"] = feats[c * BL : (c + 1) * BL]
        in_maps.append(m)

    res = bass_utils.run_bass_kernel_spmd(
        nc, in_maps, core_ids=list(range(NCORES)), trace=trace
    )
    out = np.concatenate(
        [
            np.asarray(res.results[c]["outp"], np.float32).T.reshape(BL)
            for c in range(NCORES)
        ]
    )
    return out, res


def kernel(input_features, transitions):
    out, _ = _run(input_features, transitions, trace=False)
    return out


# revision 3
# speedup vs baseline: 1592349.0000x; 147434.0000x over previous
"""Trainium2 Bass kernel for linear-chain CRF forward algorithm (log partition).

Bidirectional probability-space scan: the sequence is split at t=256;
a forward chain computes a = prod_{t=255..0}(diag(E_t) W) e_start and a
backward chain computes b = W^T diag(E_256) ... W^T diag(E_511) wstop;
log Z = log(sum_i a_i b_i) + renorm-log accumulators + S*MU.

Per core (256 batches): state tiles [128, 64] bf16, 4 partition bands of
32 tags; band r, column j <-> batch 4j+r. Per global step both chains do
one PE matmul (block-diag exp(transitions), resident-reloaded bf16
stationary) and one DVE multiply with the emission factor (PSUM -> SBUF).

Emissions stream in as natural-layout cast-DMAs (f32 HBM -> bf16 SBUF,
[t, c, i]), are transposed on PE via packed identity matmuls (stationary =
[128 t, 4 batches x 32 tags] slab -> [128, 128] PSUM tile), and
exponentiated by ACT on the PSUM->SBUF copy ([128, 512] instructions,
bias = -MU). Chunks 0 and 3 transpose during the DMA startup window;
chunks 1 and 2 are paced into the scan's PE dead time in phase 0.
"""

import os
import sys
import numpy as np

for _p in ("/opt/trn_rl_repo",):
    if _p not in sys.path and os.path.isdir(_p):
        sys.path.insert(0, _p)

import ml_dtypes

B, S, T = 2048, 512, 32
START_TAG, STOP_TAG = 30, 31
NCORES = 8
BL = B // NCORES          # 256 batch rows per core
G = 4                     # partition bands
C = BL // G               # 64 state columns (batch quads)
CHUNK = 128
NCHUNK = S // CHUNK       # 4
HALF = S // 2             # 256 scan steps per chain
MU = 4.4
FWD_RENORM = (64, 192)
BWD_RENORM = (96, 224)

_cache = {}


def _build_program(nphase=2):
    from concourse import bass, mybir
    half = 128 * nphase
    nck = 2 * nphase

    f32 = mybir.dt.float32
    bf16 = mybir.dt.bfloat16
    AF = mybir.ActivationFunctionType

    nc = bass.Bass("TRN2", target_bir_lowering=False, debug=False)

    emis = nc.dram_tensor("emis", [BL, S, T], f32, kind="ExternalInput").ap()
    wf_d = nc.dram_tensor("wf", [128, 128], bf16, kind="ExternalInput").ap()
    wb_d = nc.dram_tensor("wb", [128, 128], bf16, kind="ExternalInput").ap()
    wstart_d = nc.dram_tensor("wstart", [128, 1], f32, kind="ExternalInput").ap()
    wstop_d = nc.dram_tensor("wstop", [128, 1], f32, kind="ExternalInput").ap()
    ones_d = nc.dram_tensor("onesbd", [128, G], bf16, kind="ExternalInput").ap()
    ind_d = nc.dram_tensor("ind", [G, 128], f32, kind="ExternalInput").ap()
    ident_d = nc.dram_tensor("ident", [128, 128], bf16, kind="ExternalInput").ap()
    bmu_d = nc.dram_tensor("bmu", [128, 1], f32, kind="ExternalInput").ap()
    z4_d = nc.dram_tensor("z4", [G, 1], f32, kind="ExternalInput").ap()
    outp = nc.dram_tensor("outp", [G, C], f32, kind="ExternalOutput").ap()

    # SBUF
    wf_s = nc.alloc_sbuf_tensor("wf_s", [128, 128], bf16).ap()
    wb_s = nc.alloc_sbuf_tensor("wb_s", [128, 128], bf16).ap()
    wstart_s = nc.alloc_sbuf_tensor("wstart_s", [128, 1], f32).ap()
    wstop_s = nc.alloc_sbuf_tensor("wstop_s", [128, 1], f32).ap()
    ones_s = nc.alloc_sbuf_tensor("ones_s", [128, G], bf16).ap()
    ind_s = nc.alloc_sbuf_tensor("ind_s", [G, 128], f32).ap()
    ident_s = nc.alloc_sbuf_tensor("ident_s", [128, 128], bf16).ap()
    bmu_s = nc.alloc_sbuf_tensor("bmu_s", [128, 1], f32).ap()
    z4_s = nc.alloc_sbuf_tensor("z4_s", [G, 1], f32).ap()
    nat = [
        nc.alloc_sbuf_tensor(f"nat{i}", [128, BL, T], bf16).ap() for i in range(2)
    ]
    em = [
        nc.alloc_sbuf_tensor(f"em{k}", [128, CHUNK, C], bf16).ap()
        for k in range(NCHUNK)
    ]
    Pf = [nc.alloc_sbuf_tensor(f"Pf{i}", [128, C], bf16).ap() for i in range(2)]
    Pb = [nc.alloc_sbuf_tensor(f"Pb{i}", [128, C], bf16).ap() for i in range(2)]
    acc = nc.alloc_sbuf_tensor("acc", [G, C], f32).ap()
    ls_s = nc.alloc_sbuf_tensor("ls_s", [G, C], f32).ap()
    v_s = nc.alloc_sbuf_tensor("v_s", [G, C], f32).ap()
    tdot = nc.alloc_sbuf_tensor("tdot", [128, C], bf16).ap()
    res_s = nc.alloc_sbuf_tensor("res_s", [G, C], f32).ap()

    # PSUM (8 banks): 2 transpose banks + 4 scan q tiles + renorm pair
    trp = [
        nc.alloc_psum_tensor(f"trp{i}", [128, 4, CHUNK], f32).ap() for i in range(2)
    ]
    qf = [nc.alloc_psum_tensor(f"qf{i}", [128, C], f32).ap() for i in range(2)]
    qb = [nc.alloc_psum_tensor(f"qb{i}", [128, C], f32).ap() for i in range(2)]
    s_ps = nc.alloc_psum_tensor("s_ps", [G, C], f32).ap()
    bc_ps = nc.alloc_psum_tensor("bc_ps", [128, C], f32).ap()

    consts = [
        (wf_s, wf_d), (wb_s, wb_d), (wstart_s, wstart_d), (wstop_s, wstop_d),
        (ones_s, ones_d), (ind_s, ind_d), (ident_s, ident_d), (bmu_s, bmu_d),
        (z4_s, z4_d),
    ]
    CSEM_ALL = 16 * len(consts)

    # chunk processing order (DMA / transpose / copy): 0, 3, 1, 2
    CORDER = (0, 3, 1, 2)
    NQTR = 4              # DMA transfers per chunk (64-batch quarters)
    NJ = C                # 64 transpose matmuls per chunk (4-batch quads)
    NCP = NJ // 4         # 16 ACT copies per chunk

    # natural-buffer index per chunk
    natof = {0: 0, 3: 1, 1: 1, 2: 0}

    # transpose schedule during scan: chunk 1 at steps 25..88 (1/step),
    # chunk 2 at steps 89..120 (2/step)
    def scan_tr_count(t):
        if 25 <= t <= 88:
            return 1
        if 89 <= t <= 120:
            return 2
        return 0

    # renorm event ordering: (step, chain): ss/lss/vs/bs event index e=1..4
    fwd_rn = tuple(t for t in FWD_RENORM if t < 128 * nphase)
    bwd_rn = tuple(t for t in BWD_RENORM if t < 128 * nphase)
    EVENTS = sorted([(t, "f") for t in fwd_rn] + [(t, "b") for t in bwd_rn])
    EIDX = {ev: i + 1 for i, ev in enumerate(EVENTS)}

    def off_f(t):
        return sum(1 for te in fwd_rn if te + 4 < t)

    def off_b(t):
        return sum(1 for te in bwd_rn if te + 4 < t)

    import contextlib
    with contextlib.ExitStack() as st:
        csem = st.enter_context(nc.semaphore("csem"))
        ldsems = {
            (k, q): st.enter_context(nc.semaphore(f"ld{k}_{q}"))
            for k in range(NCHUNK)
            for q in range(NQTR)
        }
        trq = st.enter_context(nc.semaphore("trq"))
        cps = st.enter_context(nc.semaphore("cps"))
        qsf = st.enter_context(nc.semaphore("qsf"))
        psf = st.enter_context(nc.semaphore("psf"))
        qsb = st.enter_context(nc.semaphore("qsb"))
        psb = st.enter_context(nc.semaphore("psb"))
        ss = st.enter_context(nc.semaphore("ss"))
        lss = st.enter_context(nc.semaphore("lss"))
        vs = st.enter_context(nc.semaphore("vs"))
        bs = st.enter_context(nc.semaphore("bs"))
        ds = st.enter_context(nc.semaphore("ds"))
        fs = st.enter_context(nc.semaphore("fs"))
        osem = st.enter_context(nc.semaphore("osem"))

        with nc.Block() as blk:

            @blk.sync
            def _(e):
                for sb, dr in consts:
                    e.dma_start(out=sb, in_=dr).then_inc(csem, 16)
                e.wait_ge(fs, 1)
                e.dma_start(out=outp, in_=res_s).then_inc(osem, 16)
                e.wait_ge(osem, 16)

            @blk.gpsimd
            def _(e):
                # cast-DMAs f32 -> bf16, natural [t, c, i] layout.
                # startup chunks 0,3 interleaved by quarter; then 1, then 2.
                def xfer(k, q):
                    c0 = q * 64
                    e.dma_start(
                        out=nat[natof[k]][:, c0 : c0 + 64, :],
                        in_=emis[
                            c0 : c0 + 64, k * CHUNK : (k + 1) * CHUNK, :
                        ].rearrange("c t i -> t c i"),
                    ).then_inc(ldsems[(k, q)], 16)

                for q in range(NQTR):
                    xfer(0, q)
                    xfer(3, q)
                if nphase == 2:
                    # chunk 1 overwrites nat[1] (chunk 3): wait its transposes
                    e.wait_ge(trq, 2 * NJ)
                    for q in range(NQTR):
                        xfer(1, q)
                    # chunk 2 overwrites nat[0]: transposes done earlier
                    for q in range(NQTR):
                        xfer(2, q)

            # global transpose counter layout: chunk CORDER[ci] occupies
            # trq range [ci*NJ, (ci+1)*NJ); copy i covers trq 4(i+1)
            @blk.tensor
            def _(e):
                def transpose_mm(ci, k, j):
                    jj = ci * NJ + j
                    if j % 16 == 0 and jj >= 8:
                        # bank-reuse wait can't share the instruction with
                        # the ld wait: carry it on a nop
                        e.nop()._wait_ge(cps, jj // 4 - 1)
                    inst = e.matmul(
                        trp[(jj // 4) % 2][:, j % 4, :],
                        nat[natof[k]][:, 4 * j : 4 * j + 4, :],
                        ident_s,
                        start=True,
                        stop=True,
                    )
                    if j % 16 == 0:
                        inst._wait_ge(ldsems[(k, j // 16)], 16)
                    elif j % 4 == 0 and jj >= 8:
                        # bank reuse: ACT copy of group jj//4 - 2 done
                        inst._wait_ge(cps, jj // 4 - 1)
                    inst.then_inc(trq, 1)

                e.wait_ge(csem, CSEM_ALL)
                # startup: chunks 0 and 3
                for ci, k in enumerate(CORDER[:2]):
                    for j in range(NJ):
                        transpose_mm(ci, k, j)

                # scan: fwd t = 1..255 matmuls, bwd tau = 1..256
                ti = 0          # 0..127 scan-interleaved transposes (c1, c2)
                for t in range(1, half + 1):
                    if t < half:
                        e.matmul(
                            qf[t % 2], wf_s, Pf[t % 2], start=True, stop=True
                        )._wait_ge(psf, t + off_f(t)).then_inc(qsf, 1)
                    e.matmul(
                        qb[t % 2], wb_s, Pb[t % 2], start=True, stop=True
                    )._wait_ge(psb, t + off_b(t)).then_inc(qsb, 1)
                    for _ in range(scan_tr_count(t) if nphase == 2 else 0):
                        ci = 2 + ti // NJ
                        transpose_mm(ci, CORDER[ci], ti % NJ)
                        ti += 1
                    ev = (
                        (t, "f") if t in fwd_rn
                        else (t, "b") if t in bwd_rn
                        else None
                    )
                    if ev is not None:
                        ee = EIDX[ev]
                        P = Pf if ev[1] == "f" else Pb
                        if ev[1] == "f":
                            sem, tgt = psf, t + 1 + off_f(t + 1)
                        else:
                            sem, tgt = psb, t + 1 + off_b(t + 1)
                        e.matmul(
                            s_ps, ones_s, P[(t + 1) % 2], start=True, stop=True
                        )._wait_ge(sem, tgt).then_inc(ss, 1)
                    evd = (
                        (t - 3, "f") if (t - 3) in fwd_rn
                        else (t - 3, "b") if (t - 3) in bwd_rn
                        else None
                    )
                    if evd is not None:
                        e.matmul(
                            bc_ps, ind_s, v_s, start=True, stop=True
                        )._wait_ge(vs, EIDX[evd]).then_inc(bs, 1)
                assert nphase == 1 or ti == 2 * NJ, ti
                # finale: dot reduction
                e.matmul(
                    s_ps, ones_s, tdot, start=True, stop=True
                )._wait_ge(ds, 1).then_inc(ss, 1)

            @blk.scalar
            def _(e):
                def cp(i, k):
                    # copy i covers trp group i (4 j's) of chunk k
                    b = (i % NCP) * 4
                    e.activation(
                        em[k][:, :, b : b + 4],
                        trp[i % 2].rearrange("p j t -> p t j"),
                        AF.Exp,
                        bias=bmu_s,
                    )._wait_ge(trq, 4 * (i + 1)).then_inc(cps, 1)

                def renorm_act(ee):
                    e.activation(ls_s, s_ps, AF.Ln, bias=z4_s)._wait_ge(
                        ss, ee
                    ).then_inc(lss, 1)
                    e.activation(
                        v_s, ls_s, AF.Exp, bias=z4_s, scale=-1.0
                    )._wait_ge(lss, ee).then_inc(vs, 1)

                e.wait_ge(csem, CSEM_ALL)
                # copies: chunks 0,3 (i=0..31), then 1,2 interleaved with
                # renorm events by expected completion order
                for i in range(2 * NCP):
                    cp(i, CORDER[i // NCP])
                # chunk 1 copies: transposes at steps 25..88 -> trq crosses
                # 4(i+1) around step ~25+4*(i-32)... events f64 ~ after copy
                # i=41, b96 ~ after copy 47 (end of chunk 1)
                if nphase == 2:
                    for i in range(2 * NCP, 3 * NCP):
                        cp(i, CORDER[2])
                        if i == 2 * NCP + 9:
                            renorm_act(1)       # fwd t=64
                    for i in range(3 * NCP, 4 * NCP):
                        cp(i, CORDER[3])
                        if i == 3 * NCP + 3:
                            renorm_act(2)       # bwd t=96
                    renorm_act(3)               # fwd t=192
                    renorm_act(4)               # bwd t=224
                else:
                    renorm_act(1)
                    renorm_act(2)
                # finale
                e.activation(ls_s, s_ps, AF.Ln, bias=z4_s)._wait_ge(
                    ss, len(EVENTS) + 1
                ).then_inc(lss, 1)

            @blk.vector
            def _(e):
                e.wait_ge(csem, CSEM_ALL)
                e.memset(acc, 0.0)
                # init: a_1 = E_0 * wstart ; u_511 = E_511 * wstop
                e.tensor_scalar_mul(
                    Pf[1], em[0][:, 0, :], wstart_s
                )._wait_ge(cps, NCP).then_inc(psf, 1)
                e.tensor_scalar_mul(
                    Pb[1], em[3][:, CHUNK - 1, :], wstop_s
                )._wait_ge(cps, 2 * NCP).then_inc(psb, 1)

                for t in range(1, half):
                    evd = (
                        (t - 4, "f") if (t - 4) in fwd_rn
                        else (t - 4, "b") if (t - 4) in bwd_rn
                        else None
                    )
                    # fwd multiply, consumes E_t
                    k, tl = t // CHUNK, t % CHUNK
                    if t == CHUNK and nphase == 2:
                        e.nop()._wait_ge(cps, 3 * NCP)
                    instf = e.tensor_mul(
                        Pf[(t + 1) % 2], qf[t % 2], em[k][:, tl, :]
                    )
                    instf._wait_ge(qsf, t)
                    # bwd multiply, consumes E_{511-t}
                    tb = S - 1 - t
                    kb, tlb = tb // CHUNK, tb % CHUNK
                    if t == CHUNK and nphase == 2:
                        e.nop()._wait_ge(cps, 4 * NCP)
                    instb = e.tensor_mul(
                        Pb[(t + 1) % 2], qb[t % 2], em[kb][:, tlb, :]
                    )
                    instb._wait_ge(qsb, t)
                    instf.then_inc(psf, 1)
                    instb.then_inc(psb, 1)
                    if evd is not None:
                        ee = EIDX[evd]
                        if evd[1] == "f":
                            P, sem, tgt = Pf, psf, t + 1 + off_f(t)
                        else:
                            P, sem, tgt = Pb, psb, t + 1 + off_b(t)
                        # deferred rescale (normalization commutes through
                        # the linear steps in between); the nop carries the
                        # mult->rescale same-engine RAW edge
                        e.tensor_add(acc, acc, ls_s)._wait_ge(lss, ee)
                        e.nop()._wait_ge(sem, tgt)
                        e.tensor_mul(
                            P[(t + 1) % 2], bc_ps, P[(t + 1) % 2]
                        )._wait_ge(bs, ee).then_inc(sem, 1)
                # bwd final multiply tau=255 consumed E_256 above; one more
                # matmul (tau=256) produced b in qb[0]; fwd state a in Pf[0]
                e.tensor_mul(tdot, qb[0], Pf[0])._wait_ge(qsb, half).then_inc(
                    ds, 1
                )
                e.scalar_tensor_tensor(
                    res_s, ls_s, float(S) * MU, acc,
                    mybir.AluOpType.add, mybir.AluOpType.add,
                )._wait_ge(lss, len(EVENTS) + 1).then_inc(fs, 1)

    return nc


def _host_consts(transitions):
    tr = np.asarray(transitions, np.float32)
    W = np.exp(tr)                      # W[i, j] = exp(trans[i, j])
    lhsT_f = W.T.copy()                 # fwd stationary block: out = W @ P
    lhsT_b = W.copy()                   # bwd stationary block: out = W.T @ u
    wf = np.zeros((128, 128), np.float32)
    wb = np.zeros((128, 128), np.float32)
    ones_bd = np.zeros((128, G), np.float32)
    ind = np.zeros((G, 128), np.float32)
    for g in range(G):
        sl = slice(32 * g, 32 * (g + 1))
        wf[sl, sl] = lhsT_f
        wb[sl, sl] = lhsT_b
        ones_bd[sl, g] = 1.0
        ind[g, sl] = 1.0
    wstart = np.tile(np.exp(tr[:, START_TAG]), G).reshape(128, 1)
    wstop = np.tile(np.exp(tr[STOP_TAG, :]), G).reshape(128, 1)
    bf = ml_dtypes.bfloat16
    return {
        "wf": wf.astype(bf),
        "wb": wb.astype(bf),
        "wstart": wstart.astype(np.float32),
        "wstop": wstop.astype(np.float32),
        "onesbd": ones_bd.astype(bf),
        "ind": ind.astype(np.float32),
        "ident": np.eye(128, dtype=np.float32).astype(bf),
        "bmu": np.full((128, 1), -MU, np.float32),
        "z4": np.zeros((G, 1), np.float32),
    }


def _run(input_features, transitions, trace=False, nphase=2):
    from concourse import bass_utils

    feats = np.ascontiguousarray(np.asarray(input_features, np.float32))
    consts = _host_consts(transitions)

    key = f"nc{nphase}"
    if key not in _cache:
        _cache[key] = _build_program(nphase)
    nc = _cache[key]

    in_maps = []
    for c in range(NCORES):
        m = dict(consts)
        m["emis"] = feats[c * BL : (c + 1) * BL]
        in_maps.append(m)

    res = bass_utils.run_bass_kernel_spmd(
        nc, in_maps, core_ids=list(range(NCORES)), trace=trace
    )
    out = np.concatenate(
        [
            np.asarray(res.results[c]["outp"], np.float32).T.reshape(BL)
            for c in range(NCORES)
        ]
    )
    return out, res


def kernel(input_features, transitions):
    out, _ = _run(input_features, transitions, trace=False)
    return out
